# revision 10
# baseline (speedup 1.0000x reference)
"""Trainium2 Bass kernel for NestedNERModule (joint CRF loss over N*Lb lanes).

Strategy (data-parallel over docs, 8 docs per core):
  logits = embeds @ W.T + bias           -> PE matmul (fp16 in, fp32 acc)
  free CRF logZ: the BIOUL transition structure collapses the 5-state
  forward recursion to a 2-state linear recursion
      [s1,s2](t) = [s1,s2](t-1) @ F(t),  F(t) = [[EO+EU, EB],[EL, EI]](t)
  with E = exp(logits+bias), s1 = Z_O+Z_L+Z_U, s2 = Z_B+Z_I, and
      logZ = log( (F(0) @ F(1) @ ... @ F(511))_11 ).
  The 512-matrix chain product is computed as a binary tree (9 levels),
  each level = 3 fused DVE ops over a [128, 2, 2, n] broadcast AP (the
  2x2 matrix product for all lanes/entries at once), with max-rescaling
  after levels 3/5/7 (log-scales accumulated via activation accum_out).
  constrained CRF logZ: the -10000 masking collapses (exactly, in fp32)
  to the gold-path score sum_t logits[t, tag_t], computed on PE as a
  matmul against the one-hot tag mask, plus a host-side bias correction.

  Inputs are staged fp16 from host (embeds cast, W pre-transposed) to
  halve DMA bytes and skip on-device casts; embeds are transposed to
  [D, tok] on the PE (fp16 identity transposes), with PSUM->SBUF copies
  split across DVE and Act engines.
"""

import os
import sys

import numpy as np

sys.path.insert(0, "/opt/trn_rl_repo")

NUM_TAGS = 5
O_, I_, B_, L_, U_ = 0, 1, 2, 3, 4
IMPOSSIBLE = -10000.0

N_CORES = 8
N, T, D, Lb = 64, 512, 1024, 32
K = Lb * NUM_TAGS  # 160
DPC = N // N_CORES  # 8 docs per core
TT = T // 128  # 4 token tiles per doc
DC = D // 128  # 8 contraction chunks
GRPS = 2  # doc groups per core (4 docs x 32 labels = 128 lanes)
DPG = DPC // GRPS  # 4 docs per group

_CACHE = {}


def _ensure_axon_hooks_module():
    """The trn_rl_repo bass_utils imports antenv.axon_hooks when tracing;
    some images lack it.  Provide a minimal registry so trace=True degrades
    gracefully (or works, if a real hook is registered by the caller)."""
    try:
        import antenv.axon_hooks  # noqa: F401
        return
    except ImportError:
        pass
    import types

    try:
        import antenv
    except ImportError:
        return
    m = types.ModuleType("antenv.axon_hooks")
    m._hook = None

    def set_axon_ntff_profile_hook(h):
        m._hook = h

    def get_axon_ntff_profile_hook():
        return m._hook

    m.set_axon_ntff_profile_hook = set_axon_ntff_profile_hook
    m.get_axon_ntff_profile_hook = get_axon_ntff_profile_hook
    sys.modules["antenv.axon_hooks"] = m
    antenv.axon_hooks = m


# ---------------------------------------------------------------------------
# host helpers
# ---------------------------------------------------------------------------

def _build_tags(spans, n_samples, n_labels, n_tokens):
    """numpy replica of _spans_to_tags (scatter-max of BIOUL patterns)."""
    spans = np.asarray(spans)
    doc, lbl, b, e = (spans[:, i].astype(np.int64) for i in range(4))
    tags = np.zeros((n_samples, n_labels, n_tokens), np.int32)
    lengths = e - b
    for ln in np.unique(lengths):
        m = lengths == ln
        if ln <= 0:
            continue
        d_, l_, b_ = doc[m], lbl[m], b[m]
        if ln == 1:
            np.maximum.at(tags, (d_, l_, b_), U_)
        else:
            np.maximum.at(tags, (d_, l_, b_), B_)
            np.maximum.at(tags, (d_, l_, b_ + ln - 1), L_)
            for off in range(1, ln - 1):
                np.maximum.at(tags, (d_, l_, b_ + off), I_)
    return tags


def _np_lse(x, axis=-1):
    m = np.max(x, axis=axis, keepdims=True)
    return (m + np.log(np.sum(np.exp(x - m), axis=axis, keepdims=True))).squeeze(axis)


def _transitions_np():
    allowed = np.zeros((5, 5), dtype=bool)
    allowed[O_, [O_, B_, U_]] = True
    allowed[I_, [I_, L_]] = True
    allowed[B_, [I_, L_]] = True
    allowed[L_, [O_, B_, U_]] = True
    allowed[U_, [O_, B_, U_]] = True
    trans = np.where(allowed, 0.0, IMPOSSIBLE).astype(np.float32)
    start = np.where(np.array([True, False, True, False, True]), 0.0, IMPOSSIBLE).astype(np.float32)
    end = np.where(np.array([True, False, False, True, True]), 0.0, IMPOSSIBLE).astype(np.float32)
    return trans, start, end


def _crf_logz_np(logits, mask, trans, start, end):
    alpha = start[None, :] + logits[:, 0]
    for t in range(1, logits.shape[1]):
        new = _np_lse(alpha[:, :, None] + trans[None, :, :], axis=1) + logits[:, t]
        alpha = np.where(mask[:, t][:, None], new, alpha)
    return _np_lse(alpha + end[None, :], axis=-1)


def _reference_np(embeds, mask, spans, W, bias):
    """Exact numpy fallback replicating reference.py (slow; safety net only)."""
    embeds = np.asarray(embeds, np.float32)
    mask = np.asarray(mask, bool)
    W = np.asarray(W, np.float32)
    bias = np.asarray(bias, np.float32)
    n, t, d = embeds.shape
    n_labels = W.shape[0] // NUM_TAGS
    trans, start, end = _transitions_np()
    logits = np.einsum("ntd,kd->ntk", embeds, W) + bias
    crf_logits = (
        logits.reshape(n, t, n_labels, NUM_TAGS)
        .transpose(0, 2, 1, 3)
        .reshape(n * n_labels, t, NUM_TAGS)
    )
    crf_mask = np.repeat(mask, n_labels, axis=0)
    tags = _build_tags(spans, n, n_labels, t)
    target = np.eye(NUM_TAGS, dtype=bool)[tags].reshape(n * n_labels, t, NUM_TAGS)
    clogits = np.where(target, crf_logits, np.float32(IMPOSSIBLE))
    per_seq = _crf_logz_np(crf_logits, crf_mask, trans, start, end) - _crf_logz_np(
        clogits, crf_mask, trans, start, end
    )
    invalid = np.any(per_seq > -IMPOSSIBLE)
    loss = np.float32(0.0) if invalid else per_seq.sum(dtype=np.float32)
    return np.array([loss / 100.0], dtype=np.float32)


def _gold_path_valid(tags):
    """Check every lane's tag sequence is a legal BIOUL path (start/trans/end)."""
    allowed = np.zeros((5, 5), dtype=bool)
    allowed[O_, [O_, B_, U_]] = True
    allowed[I_, [I_, L_]] = True
    allowed[B_, [I_, L_]] = True
    allowed[L_, [O_, B_, U_]] = True
    allowed[U_, [O_, B_, U_]] = True
    start_ok = np.isin(tags[..., 0], [O_, B_, U_]).all()
    end_ok = np.isin(tags[..., -1], [O_, L_, U_]).all()
    trans_ok = allowed[tags[..., :-1], tags[..., 1:]].all()
    return bool(start_ok and end_ok and trans_ok)


# ---------------------------------------------------------------------------
# bass program
# ---------------------------------------------------------------------------

def _build_bass():
    import concourse.bacc as bacc
    import concourse.mybir as mybir
    import concourse.tile as tile
    from concourse.masks import make_identity

    f32 = mybir.dt.float32
    f16 = mybir.dt.float16
    AF = mybir.ActivationFunctionType
    ALU = mybir.AluOpType
    AX = mybir.AxisListType

    nc = bacc.Bacc()
    emb_h = nc.declare_dram_parameter("emb", [DPC, T, D], f16, isOutput=False)
    wt_h = nc.declare_dram_parameter("wt", [D, K], f16, isOutput=False)
    oh_h = nc.declare_dram_parameter("oh5", [DPC, T, NUM_TAGS], f16, isOutput=False)
    biasg_h = nc.declare_dram_parameter("biasg", [128, NUM_TAGS], f32, isOutput=False)
    logz_h = nc.declare_dram_parameter("logz", [128, GRPS], f32, isOutput=True)
    golds_h = nc.declare_dram_parameter("golds", [NUM_TAGS, DPC, K], f32, isOutput=True)

    with tile.TileContext(nc) as tc:
        with (
            tc.tile_pool(name="const", bufs=1) as constp,
            tc.tile_pool(name="embtp", bufs=3) as embtp,
            tc.tile_pool(name="lgtp", bufs=8) as lgtp,
            tc.tile_pool(name="ohp", bufs=3) as ohp,
            tc.tile_pool(name="treep", bufs=1) as treep,
            tc.tile_pool(name="pl", bufs=4, space="PSUM") as pl,
            tc.tile_pool(name="pg", bufs=2, space="PSUM") as pg,
            tc.tile_pool(name="pgold", bufs=2, space="PSUM") as pgold,
        ):
            identity16 = constp.tile([128, 128], f16)
            make_identity(nc, identity16[:])

            # host-pretransposed W: [D, K] fp16 -> [128, DC, K]
            wt = constp.tile([128, DC, K], f16)
            nc.sync.dma_start(wt[:], wt_h.rearrange("(dc p) k -> p dc k", p=128))

            bias_sb = constp.tile([128, NUM_TAGS], f32)
            nc.sync.dma_start(bias_sb[:], biasg_h[:])

            logz_sb = constp.tile([128, GRPS], f32)
            golds_sb = constp.tile([NUM_TAGS, DPC, K], f32)

            lgw = [[None] * TT for _ in range(GRPS)]

            # --- CRF tree over one group's 128 lanes -------------------------
            # cur tile layout: [128, 4, n] fp32, plane ij: 0=(11)=EO+EU,
            # 1=(12)=EB, 2=(21)=EL, 3=(22)=EI; positions innermost.
            def emit_tree(grp, cur):
                laccs = []
                tmp = treep.tile([128, 4, 256], f32, tag=f"tmp{grp}", name=f"tmp{grp}")
                for lvl in range(1, 10):
                    n = T >> lvl
                    X = cur[:]
                    # A[i][k] even positions, B[k][j] odd positions
                    A1 = X[:, 0:3:2, 0::2].unsqueeze(2).broadcast_to([128, 2, 2, n])
                    B1 = X[:, 0:2, 1::2].unsqueeze(1).broadcast_to([128, 2, 2, n])
                    A2 = X[:, 1:4:2, 0::2].unsqueeze(2).broadcast_to([128, 2, 2, n])
                    B2 = X[:, 2:4, 1::2].unsqueeze(1).broadcast_to([128, 2, 2, n])
                    Y = treep.tile([128, 4, n], f32, tag=f"c{grp}_{lvl}", name=f"c{grp}_{lvl}")
                    Y4 = Y[:].rearrange("p (i j) n -> p i j n", i=2)
                    t4 = tmp[:, :, 0:n].rearrange("p (i j) n -> p i j n", i=2)
                    nc.vector.tensor_mul(t4, A1, B1)
                    nc.vector.tensor_mul(Y4, A2, B2)
                    nc.vector.tensor_add(Y4, Y4, t4)
                    cur = Y
                    if lvl in (3, 5, 7):
                        m = treep.tile([128, n], f32, tag=f"m{grp}_{lvl}", name=f"m{grp}_{lvl}")
                        nc.vector.tensor_reduce(
                            m[:], Y[:].rearrange("p a n -> p n a"), AX.X, ALU.max
                        )
                        r = treep.tile([128, n], f32, tag=f"r{grp}_{lvl}", name=f"r{grp}_{lvl}")
                        nc.vector.reciprocal(r[:], m[:])
                        nc.vector.tensor_mul(
                            Y[:], Y[:], r[:].unsqueeze(1).broadcast_to([128, 4, n])
                        )
                        lnm = treep.tile([128, n], f32, tag=f"ln{grp}_{lvl}", name=f"ln{grp}_{lvl}")
                        lacc = treep.tile([128, 1], f32, tag=f"la{grp}_{lvl}", name=f"la{grp}_{lvl}")
                        nc.scalar.activation(lnm[:], m[:], AF.Ln, accum_out=lacc[:])
                        laccs.append(lacc)
                    yield lvl
                # logZ = ln(C11_final) + sum of log-scales
                lnp = treep.tile([128, 1], f32, tag=f"lnp{grp}", name=f"lnp{grp}")
                nc.scalar.activation(lnp[:], cur[:, 0, :], AF.Ln)
                acc = treep.tile([128, 1], f32, tag=f"acc{grp}", name=f"acc{grp}")
                nc.vector.tensor_add(acc[:], laccs[0][:], laccs[1][:])
                nc.vector.tensor_add(acc[:], acc[:], laccs[2][:])
                nc.vector.tensor_add(logz_sb[:, grp : grp + 1], acc[:], lnp[:])
                yield 10

            # --- per-group plane transposes + exp ---------------------------
            def emit_planes(grp):
                cur = treep.tile([128, 4, T], f32, tag=f"cur{grp}", name=f"cur{grp}")
                scr = treep.tile([128, T], f32, tag=f"scr{grp}", name=f"scr{grp}")
                # tag g -> cur plane: O->0, U->scratch (added into 0), B->1,
                # L->2, I->3
                plane = {O_: 0, B_: 1, L_: 2, I_: 3}
                for g in range(NUM_TAGS):
                    pgt = pg.tile([128, 512], f16, tag="pg")
                    for tt in range(TT):
                        nc.tensor.transpose(
                            pgt[:, tt * 128 : (tt + 1) * 128],
                            lgw[grp][tt][:, :, g::NUM_TAGS],
                            identity16[:],
                        )
                    dst = scr[:] if g == U_ else cur[:, plane[g], :]
                    nc.scalar.activation(
                        dst, pgt[:], AF.Exp, bias=bias_sb[:, g : g + 1]
                    )
                nc.vector.tensor_add(cur[:, 0, :], cur[:, 0, :], scr[:])
                return cur

            # --- main per-doc pipeline --------------------------------------
            pending_gold = None  # (doc, oh_tile, ...) emitted one doc late
            pending_planes = None  # group whose planes/tree start next doc
            tree_gen = None

            def emit_gold(d, oh, lgw_g, dd):
                pgold_t = pgold.tile([NUM_TAGS, K], f32, tag="pgold")
                for tt in range(TT):
                    nc.tensor.matmul(
                        pgold_t[:],
                        oh[:, tt, :],
                        lgw_g[tt][:, dd, :],
                        start=(tt == 0),
                        stop=(tt == TT - 1),
                    )
                nc.vector.tensor_copy(golds_sb[:, d, :], pgold_t[:])

            for d in range(DPC):
                grp, dd = divmod(d, DPG)

                # XBAR dma-transpose straight from DRAM:
                # embt[p, dc, t] = emb[t, dc*128 + p], fp16
                embt = embtp.tile([128, DC, T], f16, tag="embt")
                nc.sync.dma_start_transpose(embt[:], emb_h[d])

                oh = ohp.tile([128, TT, NUM_TAGS], f16, tag="oh")
                nc.sync.dma_start(oh[:], oh_h[d].rearrange("(a p) g -> p a g", p=128))

                # gold matmuls for the previous doc (its lgw copies are done
                # by now, so PE doesn't stall on the copy engines)
                if pending_gold is not None:
                    emit_gold(*pending_gold)
                    pending_gold = None
                if pending_planes is not None:
                    cur = emit_planes(pending_planes)
                    tree_gen = emit_tree(pending_planes, cur)
                    pending_planes = None

                # matmul phase: logits [128 tok, K] per tt
                for tt in range(TT):
                    if dd == 0:
                        lgw[grp][tt] = lgtp.tile(
                            [128, DPG, K], f16, tag="lg", name=f"lg{grp}_{tt}"
                        )
                    pl_t = pl.tile([128, K], f32, tag="pl")
                    for dc in range(DC):
                        nc.tensor.matmul(
                            pl_t[:],
                            embt[:, dc, tt * 128 : (tt + 1) * 128],
                            wt[:, dc, :],
                            start=(dc == 0),
                            stop=(dc == DC - 1),
                        )
                    lg = lgw[grp][tt][:, dd, :]
                    nc.scalar.copy(lg, pl_t[:])
                pending_gold = (d, oh, lgw[grp], dd)

                # pump the previous group's tree between docs so its DVE ops
                # interleave with this doc's copies in the DVE stream
                if tree_gen is not None:
                    for _ in range(4):
                        if next(tree_gen, None) is None:
                            tree_gen = None
                            break

                if dd == DPG - 1:
                    pending_planes = grp

            emit_gold(*pending_gold)
            if tree_gen is not None:
                for _ in tree_gen:
                    pass
            if pending_planes is not None:
                cur = emit_planes(pending_planes)
                for _ in emit_tree(pending_planes, cur):
                    pass

            nc.sync.dma_start(logz_h[:], logz_sb[:])
            nc.sync.dma_start(golds_h[:], golds_sb[:])

    nc.finalize()
    return nc


def _get_nc():
    if "nc" not in _CACHE:
        _CACHE["nc"] = _build_bass()
    return _CACHE["nc"]


# ---------------------------------------------------------------------------
# entry point
# ---------------------------------------------------------------------------

last_results = None


def kernel(embeds, mask, spans, W, bias):
    global last_results
    embeds = np.asarray(embeds)
    mask = np.asarray(mask)
    spans = np.asarray(spans)
    W = np.asarray(W, dtype=np.float32)
    bias = np.asarray(bias, dtype=np.float32)

    if embeds.shape != (N, T, D) or W.shape != (K, D) or not mask.all():
        return _reference_np(embeds, mask, spans, W, bias)

    tags = _build_tags(spans, N, Lb, T)
    # fast path requires per-doc label-independent tags and valid gold paths
    if not (tags == tags[:, :1, :]).all() or not _gold_path_valid(tags):
        return _reference_np(embeds, mask, spans, W, bias)

    # host-side input staging (casts/layout + index/mask building only)
    emb16 = np.ascontiguousarray(embeds.astype(np.float16))
    wt16 = np.ascontiguousarray(W.T.astype(np.float16))  # [D, K]
    tag_d = tags[:, 0, :]  # [N, T]
    oh5 = (tag_d[:, :, None] == np.arange(NUM_TAGS)[None, None, :]).astype(np.float16)
    p = np.arange(128)
    biasg = bias[(NUM_TAGS * (p % Lb))[:, None] + np.arange(NUM_TAGS)[None, :]]
    biasg = np.ascontiguousarray(biasg, dtype=np.float32)
    # gold bias correction: sum_t bias[5l + tag[d,l,t]]
    k_idx = (NUM_TAGS * np.arange(Lb))[None, :, None] + tags  # [N, Lb, T]
    biasgold = bias[k_idx].sum(axis=-1, dtype=np.float32)  # [N, Lb]

    _ensure_axon_hooks_module()
    from concourse.bass_utils import run_bass_kernel_spmd

    nc = _get_nc()
    in_maps = []
    for c in range(N_CORES):
        in_maps.append(
            {
                "emb": emb16[c * DPC : (c + 1) * DPC],
                "wt": wt16,
                "oh5": np.ascontiguousarray(oh5[c * DPC : (c + 1) * DPC]),
                "biasg": biasg,
            }
        )
    res = run_bass_kernel_spmd(
        nc,
        in_maps,
        list(range(N_CORES)),
        trace=bool(os.environ.get("BASS_TRACE")),
    )
    last_results = res

    logz = np.zeros((N, Lb), np.float32)
    gold = np.zeros((N, Lb), np.float32)
    for c in range(N_CORES):
        lz = np.asarray(res.results[c]["logz"])  # [128, GRPS]
        gd = np.asarray(res.results[c]["golds"])  # [5, DPC, K]
        for grp in range(GRPS):
            for dd in range(DPG):
                doc = c * DPC + grp * DPG + dd
                logz[doc] = lz[32 * dd : 32 * (dd + 1), grp]
        for dl in range(DPC):
            doc = c * DPC + dl
            # gold[l] = sum_g gd[g, dl, 5l+g]
            gold[doc] = gd[:, dl, :].reshape(NUM_TAGS, Lb, NUM_TAGS)[
                np.arange(NUM_TAGS), :, np.arange(NUM_TAGS)
            ].sum(axis=0)

    per_seq = logz - (gold + biasgold)
    invalid = np.any(per_seq > -IMPOSSIBLE)
    loss = np.float32(0.0) if invalid else per_seq.sum(dtype=np.float32)
    return np.array([loss / 100.0], dtype=np.float32)
